# revision 7
# baseline (speedup 1.0000x reference)
"""Per-row asymmetric int4 quantization (QuaRot asym_quant) on 8 TRN2 cores.

Full input x: [16384, 4096] f32. Outputs: q [16384,4096] f32,
scale [16384,1] f32, zeros [16384,1] f16.

Sharding: trivially data-parallel along rows — 2048 rows per core, no
communication. Per core: 16 tiles of [128 partitions, 4096 free].

Per-tile math (row r on partition p):
  mx = max(x_r); nm = -min(x_r)            (DVE tensor_scalar with fused
                                            max-reduce accumulator)
  d = mx + nm; scale = d * (1/15); a = 1/scale   (DVE reciprocal = HW divide)
  zb = RN(nm*a + MAGIC) = MAGIC + round(-min/scale) = MAGIC + zeros
       (ACT fma; MAGIC = 1.5*2^23 so RN lands on the integer grid, half-even
        like jnp.round)
  v  = RN(x*a + zb) = MAGIC + round(x/scale) + zeros   (ACT big pass)
  v  = min(max(v, MAGIC), MAGIC+15)        (DVE tensor_scalar, both clamps)
  q  = v - MAGIC                           (ACT, Sterbenz-exact)

DMA discipline: in-DMAs issue from the Sync sequencer, out-DMAs from the
Scalar sequencer (two separate HWDGE rings) so a blocked out-DMA never
stalls the issue of the next tile's load. scale/zeros are staged in
[128, NT] SBUF tiles and written with one DMA each at the end (DRAM
layout [P, NT], transposed to [R, 1] on the host).
"""

import numpy as np

import concourse.bacc as bacc
import concourse.bass as bass
import concourse.tile as tile
from concourse import mybir
from concourse.bass_utils import run_bass_kernel_spmd

N_CORES = 8
R_FULL, C = 16384, 4096
R = R_FULL // N_CORES  # rows per core
P = 128                # partitions per tile
NT = R // P            # tiles per core
MAXQ = 15.0
MAGIC = 12582912.0     # 1.5 * 2**23: RN(t + MAGIC) == MAGIC + round_half_even(t)

_cached = {}


def build_nc() -> bass.Bass:
    nc = bacc.Bacc("TRN2", target_bir_lowering=False)
    x = nc.dram_tensor("x", [R, C], mybir.dt.float32, kind="ExternalInput").ap()
    q = nc.dram_tensor("q", [R, C], mybir.dt.float32, kind="ExternalOutput").ap()
    # staged [P, NT]: column j = tile j's per-partition value; host transposes
    s = nc.dram_tensor("scale", [P, NT], mybir.dt.float32, kind="ExternalOutput").ap()
    z = nc.dram_tensor("zeros", [P, NT], mybir.dt.float16, kind="ExternalOutput").ap()

    xt = x.rearrange("(n p) c -> n p c", p=P)
    qt = q.rearrange("(n p) c -> n p c", p=P)

    with tile.TileContext(nc) as tc:
        with (
            tc.tile_pool(name="xp", bufs=4) as xp,
            tc.tile_pool(name="vp", bufs=3) as vp,
            tc.tile_pool(name="sm", bufs=4) as sm,
            tc.tile_pool(name="singles", bufs=1) as singles,
        ):
            magic_b = singles.tile([P, 1], mybir.dt.float32, tag="magic_b")
            nc.vector.memset(magic_b, MAGIC)
            neg_magic_b = singles.tile([P, 1], mybir.dt.float32, tag="neg_magic_b")
            nc.vector.memset(neg_magic_b, -MAGIC)
            s_acc = singles.tile([P, NT], mybir.dt.float32, tag="s_acc")
            z_acc = singles.tile([P, NT], mybir.dt.float16, tag="z_acc")
            for i in range(NT):
                xtile = xp.tile([P, C], mybir.dt.float32)
                nc.sync.dma_start(out=xtile, in_=xt[i])

                mx = sm.tile([P, 1], mybir.dt.float32, tag="mx")
                nm = sm.tile([P, 1], mybir.dt.float32, tag="nm")
                dummy = sm.tile([P, 1], mybir.dt.float32, tag="dummy")
                dummy2 = sm.tile([P, 1], mybir.dt.float32, tag="dummy2")
                # row max: tensor_scalar bypass with fused max-reduce
                nc.vector.tensor_scalar(
                    out=dummy.broadcast_to([P, C]),
                    in0=xtile,
                    scalar1=0.0,
                    scalar2=None,
                    op0=mybir.AluOpType.bypass,
                    op1=mybir.AluOpType.max,
                    accum_out=mx,
                )
                # negated row min: (x * -1) max-reduced
                nc.vector.tensor_scalar(
                    out=dummy2.broadcast_to([P, C]),
                    in0=xtile,
                    scalar1=-1.0,
                    scalar2=None,
                    op0=mybir.AluOpType.mult,
                    op1=mybir.AluOpType.max,
                    accum_out=nm,
                )
                d = sm.tile([P, 1], mybir.dt.float32, tag="d")
                nc.vector.tensor_tensor(out=d, in0=mx, in1=nm, op=mybir.AluOpType.add)
                # scale for this tile -> staging column i (on ACT)
                nc.scalar.mul(s_acc[:, i : i + 1], d, 1.0 / MAXQ)
                a = sm.tile([P, 1], mybir.dt.float32, tag="a")
                nc.vector.reciprocal(out=a, in_=s_acc[:, i : i + 1])
                # zb = MAGIC + round(nm * a) = MAGIC + zeros
                zb = sm.tile([P, 1], mybir.dt.float32, tag="zb")
                nc.scalar.activation(
                    out=zb,
                    in_=nm,
                    func=mybir.ActivationFunctionType.Identity,
                    bias=magic_b,
                    scale=a,
                )
                # zeros f16 -> staging column i (on ACT; f32->f16 on write)
                nc.scalar.activation(
                    out=z_acc[:, i : i + 1],
                    in_=zb,
                    func=mybir.ActivationFunctionType.Identity,
                    bias=neg_magic_b,
                    scale=1.0,
                )

                # big pass: v = RN(x*a + zb) -> MAGIC + round(x/scale) + zeros
                v = vp.tile([P, C], mybir.dt.float32)
                nc.scalar.activation(
                    out=v,
                    in_=xtile,
                    func=mybir.ActivationFunctionType.Identity,
                    bias=zb,
                    scale=a,
                )
                # both clamps in one GpSimd pass (DVE stays on reduces)
                nc.gpsimd.tensor_scalar(
                    out=v,
                    in0=v,
                    scalar1=MAGIC,
                    scalar2=MAGIC + MAXQ,
                    op0=mybir.AluOpType.max,
                    op1=mybir.AluOpType.min,
                )
                # subtract MAGIC (exact) on ACT
                nc.scalar.activation(
                    out=v,
                    in_=v,
                    func=mybir.ActivationFunctionType.Identity,
                    bias=neg_magic_b,
                    scale=1.0,
                )

                # out-DMA on the Scalar HWDGE ring (keeps Sync ring free for loads)
                nc.scalar.dma_start(out=qt[i], in_=v)
            nc.scalar.dma_start(out=s, in_=s_acc)
            nc.scalar.dma_start(out=z, in_=z_acc)
    nc.finalize()
    return nc


def _get_nc() -> bass.Bass:
    if "nc" not in _cached:
        _cached["nc"] = build_nc()
    return _cached["nc"]


def kernel(x: np.ndarray, **_unused) -> tuple[np.ndarray, np.ndarray, np.ndarray]:
    x = np.ascontiguousarray(np.asarray(x, dtype=np.float32))
    assert x.shape == (R_FULL, C), x.shape
    nc = _get_nc()
    in_maps = [
        {"x": np.ascontiguousarray(x[i * R : (i + 1) * R])} for i in range(N_CORES)
    ]
    res = run_bass_kernel_spmd(nc, in_maps, core_ids=list(range(N_CORES)))
    q = np.concatenate([res.results[i]["q"] for i in range(N_CORES)], axis=0)
    # staged [P, NT] -> [R, 1]: row j*P + p = staged[p, j]
    scale = np.concatenate(
        [res.results[i]["scale"].T.reshape(R, 1) for i in range(N_CORES)], axis=0
    )
    zeros = np.concatenate(
        [res.results[i]["zeros"].T.reshape(R, 1) for i in range(N_CORES)], axis=0
    )
    return q, scale.astype(np.float32), zeros.astype(np.float16)


# revision 8
# speedup vs baseline: 3.7720x; 3.7720x over previous
"""Per-row asymmetric int4 quantization (QuaRot asym_quant) on 8 TRN2 cores.

Full input x: [16384, 4096] f32. Outputs: q [16384,4096] f32,
scale [16384,1] f32, zeros [16384,1] f16.

Sharding: trivially data-parallel along rows — 2048 rows per core, no
communication. Per core: 16 tiles of [128 partitions, 4096 free].

Per-tile math (row r on partition p):
  mx = max(x_r); nm = -min(x_r)            (DVE tensor_scalar with fused
                                            max-reduce accumulator)
  d = mx + nm; scale = d * (1/15); a = 1/scale   (DVE reciprocal = HW divide)
  zb = RN(nm*a + MAGIC) = MAGIC + round(-min/scale) = MAGIC + zeros
       (ACT fma; MAGIC = 1.5*2^23 so RN lands on the integer grid, half-even
        like jnp.round)
  v  = RN(x*a + zb) = MAGIC + round(x/scale) + zeros   (ACT big pass)
  v  = min(max(v, MAGIC), MAGIC+15)        (DVE tensor_scalar, both clamps)
  q  = v - MAGIC                           (ACT, Sterbenz-exact)

DMA discipline: in-DMAs issue from the Sync sequencer, out-DMAs from the
Scalar sequencer (two separate HWDGE rings) so a blocked out-DMA never
stalls the issue of the next tile's load. scale/zeros are staged in
[128, NT] SBUF tiles and written with one DMA each at the end (DRAM
layout [P, NT], transposed to [R, 1] on the host).
"""

import numpy as np

import concourse.bacc as bacc
import concourse.bass as bass
import concourse.tile as tile
from concourse import mybir
from concourse.bass_utils import run_bass_kernel_spmd

N_CORES = 8
R_FULL, C = 16384, 4096
R = R_FULL // N_CORES  # rows per core
P = 128                # partitions per tile
NT = R // P            # tiles per core
MAXQ = 15.0
MAGIC = 12582912.0     # 1.5 * 2**23: RN(t + MAGIC) == MAGIC + round_half_even(t)

_cached = {}


def build_nc() -> bass.Bass:
    nc = bacc.Bacc("TRN2", target_bir_lowering=False)
    x = nc.dram_tensor("x", [R, C], mybir.dt.float32, kind="ExternalInput").ap()
    q = nc.dram_tensor("q", [R, C], mybir.dt.float32, kind="ExternalOutput").ap()
    # staged [P, NT]: column j = tile j's per-partition value; host transposes
    s = nc.dram_tensor("scale", [P, NT], mybir.dt.float32, kind="ExternalOutput").ap()
    z = nc.dram_tensor("zeros", [P, NT], mybir.dt.float16, kind="ExternalOutput").ap()

    xt = x.rearrange("(n p) c -> n p c", p=P)
    qt = q.rearrange("(n p) c -> n p c", p=P)

    with tile.TileContext(nc) as tc:
        with (
            tc.tile_pool(name="xp", bufs=5) as xp,
            tc.tile_pool(name="vp", bufs=4) as vp,
            tc.tile_pool(name="sm", bufs=4) as sm,
            tc.tile_pool(name="singles", bufs=1) as singles,
        ):
            magic_b = singles.tile([P, 1], mybir.dt.float32, tag="magic_b")
            nc.vector.memset(magic_b, MAGIC)
            neg_magic_b = singles.tile([P, 1], mybir.dt.float32, tag="neg_magic_b")
            nc.vector.memset(neg_magic_b, -MAGIC)
            s_acc = singles.tile([P, NT], mybir.dt.float32, tag="s_acc")
            z_acc = singles.tile([P, NT], mybir.dt.float16, tag="z_acc")
            for i in range(NT):
                xtile = xp.tile([P, C], mybir.dt.float32)
                nc.sync.dma_start(out=xtile, in_=xt[i])

                mx = sm.tile([P, 1], mybir.dt.float32, tag="mx")
                nm = sm.tile([P, 1], mybir.dt.float32, tag="nm")
                dummy = sm.tile([P, 1], mybir.dt.float32, tag="dummy")
                dummy2 = sm.tile([P, 1], mybir.dt.float32, tag="dummy2")
                # row max: tensor_scalar bypass with fused max-reduce
                nc.vector.tensor_scalar(
                    out=dummy.broadcast_to([P, C]),
                    in0=xtile,
                    scalar1=0.0,
                    scalar2=None,
                    op0=mybir.AluOpType.bypass,
                    op1=mybir.AluOpType.max,
                    accum_out=mx,
                )
                # negated row min: (x * -1) max-reduced
                nc.vector.tensor_scalar(
                    out=dummy2.broadcast_to([P, C]),
                    in0=xtile,
                    scalar1=-1.0,
                    scalar2=None,
                    op0=mybir.AluOpType.mult,
                    op1=mybir.AluOpType.max,
                    accum_out=nm,
                )
                d = sm.tile([P, 1], mybir.dt.float32, tag="d")
                nc.vector.tensor_tensor(out=d, in0=mx, in1=nm, op=mybir.AluOpType.add)
                # scale for this tile -> staging column i (on ACT)
                nc.scalar.mul(s_acc[:, i : i + 1], d, 1.0 / MAXQ)
                a = sm.tile([P, 1], mybir.dt.float32, tag="a")
                nc.vector.reciprocal(out=a, in_=s_acc[:, i : i + 1])
                # zb = MAGIC + round(nm * a) = MAGIC + zeros
                zb = sm.tile([P, 1], mybir.dt.float32, tag="zb")
                nc.scalar.activation(
                    out=zb,
                    in_=nm,
                    func=mybir.ActivationFunctionType.Identity,
                    bias=magic_b,
                    scale=a,
                )
                # zeros f16 -> staging column i (on ACT; f32->f16 on write)
                nc.scalar.activation(
                    out=z_acc[:, i : i + 1],
                    in_=zb,
                    func=mybir.ActivationFunctionType.Identity,
                    bias=neg_magic_b,
                    scale=1.0,
                )

                # big pass: v = RN(x*a + zb) -> MAGIC + round(x/scale) + zeros
                v = vp.tile([P, C], mybir.dt.float32)
                nc.scalar.activation(
                    out=v,
                    in_=xtile,
                    func=mybir.ActivationFunctionType.Identity,
                    bias=zb,
                    scale=a,
                )
                # both clamps in one DVE pass (2x mode: single-src f32 SBUF)
                nc.vector.tensor_scalar(
                    out=v,
                    in0=v,
                    scalar1=MAGIC,
                    scalar2=MAGIC + MAXQ,
                    op0=mybir.AluOpType.max,
                    op1=mybir.AluOpType.min,
                )
                # subtract MAGIC (exact) on ACT
                nc.scalar.activation(
                    out=v,
                    in_=v,
                    func=mybir.ActivationFunctionType.Identity,
                    bias=neg_magic_b,
                    scale=1.0,
                )

                # out-DMA on the Scalar HWDGE ring (keeps Sync ring free for loads)
                nc.scalar.dma_start(out=qt[i], in_=v)
            nc.scalar.dma_start(out=s, in_=s_acc)
            nc.scalar.dma_start(out=z, in_=z_acc)
    nc.finalize()
    return nc


def _get_nc() -> bass.Bass:
    if "nc" not in _cached:
        _cached["nc"] = build_nc()
    return _cached["nc"]


def kernel(x: np.ndarray, **_unused) -> tuple[np.ndarray, np.ndarray, np.ndarray]:
    x = np.ascontiguousarray(np.asarray(x, dtype=np.float32))
    assert x.shape == (R_FULL, C), x.shape
    nc = _get_nc()
    in_maps = [
        {"x": np.ascontiguousarray(x[i * R : (i + 1) * R])} for i in range(N_CORES)
    ]
    res = run_bass_kernel_spmd(nc, in_maps, core_ids=list(range(N_CORES)))
    q = np.concatenate([res.results[i]["q"] for i in range(N_CORES)], axis=0)
    # staged [P, NT] -> [R, 1]: row j*P + p = staged[p, j]
    scale = np.concatenate(
        [res.results[i]["scale"].T.reshape(R, 1) for i in range(N_CORES)], axis=0
    )
    zeros = np.concatenate(
        [res.results[i]["zeros"].T.reshape(R, 1) for i in range(N_CORES)], axis=0
    )
    return q, scale.astype(np.float32), zeros.astype(np.float16)


# revision 9
# speedup vs baseline: 3.7746x; 1.0007x over previous
"""Per-row asymmetric int4 quantization (QuaRot asym_quant) on 8 TRN2 cores.

Full input x: [16384, 4096] f32. Outputs: q [16384,4096] f32,
scale [16384,1] f32, zeros [16384,1] f16.

Sharding: trivially data-parallel along rows — 2048 rows per core, no
communication. Per core: 16 tiles of [128 partitions, 4096 free].

Per-tile math (row r on partition p):
  mx = max(x_r); nm = -min(x_r)            (DVE tensor_scalar with fused
                                            max-reduce accumulator)
  d = mx + nm; scale = d * (1/15); a = 1/scale   (DVE reciprocal = HW divide)
  zb = RN(nm*a + MAGIC) = MAGIC + round(-min/scale) = MAGIC + zeros
       (ACT fma; MAGIC = 1.5*2^23 so RN lands on the integer grid, half-even
        like jnp.round)
  v  = RN(x*a + zb) = MAGIC + round(x/scale) + zeros   (ACT big pass)
  v  = min(max(v, MAGIC), MAGIC+15)        (DVE tensor_scalar, both clamps)
  q  = v - MAGIC                           (ACT, Sterbenz-exact)

DMA discipline: in-DMAs issue from the Sync sequencer, out-DMAs from the
Scalar sequencer (two separate HWDGE rings) so a blocked out-DMA never
stalls the issue of the next tile's load. scale/zeros are staged in
[128, NT] SBUF tiles and written with one DMA each at the end (DRAM
layout [P, NT], transposed to [R, 1] on the host).
"""

import numpy as np

import concourse.bacc as bacc
import concourse.bass as bass
import concourse.tile as tile
from concourse import mybir
from concourse.bass_utils import run_bass_kernel_spmd

N_CORES = 8
R_FULL, C = 16384, 4096
R = R_FULL // N_CORES  # rows per core
P = 128                # partitions per tile
NT = R // P            # tiles per core
MAXQ = 15.0
MAGIC = 12582912.0     # 1.5 * 2**23: RN(t + MAGIC) == MAGIC + round_half_even(t)

_cached = {}


def build_nc() -> bass.Bass:
    nc = bacc.Bacc("TRN2", target_bir_lowering=False)
    x = nc.dram_tensor("x", [R, C], mybir.dt.float32, kind="ExternalInput").ap()
    q = nc.dram_tensor("q", [R, C], mybir.dt.float32, kind="ExternalOutput").ap()
    # staged [P, NT]: column j = tile j's per-partition value; host transposes
    s = nc.dram_tensor("scale", [P, NT], mybir.dt.float32, kind="ExternalOutput").ap()
    z = nc.dram_tensor("zeros", [P, NT], mybir.dt.float16, kind="ExternalOutput").ap()

    xt = x.rearrange("(n p) c -> n p c", p=P)
    qt = q.rearrange("(n p) c -> n p c", p=P)

    with tile.TileContext(nc) as tc:
        with (
            tc.tile_pool(name="xp", bufs=5) as xp,
            tc.tile_pool(name="vp", bufs=4) as vp,
            tc.tile_pool(name="sm", bufs=4) as sm,
            tc.tile_pool(name="singles", bufs=1) as singles,
        ):
            magic_b = singles.tile([P, 1], mybir.dt.float32, tag="magic_b")
            nc.vector.memset(magic_b, MAGIC)
            neg_magic_b = singles.tile([P, 1], mybir.dt.float32, tag="neg_magic_b")
            nc.vector.memset(neg_magic_b, -MAGIC)
            s_acc = singles.tile([P, NT], mybir.dt.float32, tag="s_acc")
            z_acc = singles.tile([P, NT], mybir.dt.float16, tag="z_acc")
            for i in range(NT):
                xtile = xp.tile([P, C], mybir.dt.float32)
                nc.sync.dma_start(out=xtile, in_=xt[i])

                mx = sm.tile([P, 1], mybir.dt.float32, tag="mx")
                nm = sm.tile([P, 1], mybir.dt.float32, tag="nm")
                dummy = sm.tile([P, 1], mybir.dt.float32, tag="dummy")
                dummy2 = sm.tile([P, 1], mybir.dt.float32, tag="dummy2")
                # row max: tensor_scalar bypass with fused max-reduce
                nc.vector.tensor_scalar(
                    out=dummy.broadcast_to([P, C]),
                    in0=xtile,
                    scalar1=0.0,
                    scalar2=None,
                    op0=mybir.AluOpType.bypass,
                    op1=mybir.AluOpType.max,
                    accum_out=mx,
                )
                # negated row min: (x * -1) max-reduced
                nc.vector.tensor_scalar(
                    out=dummy2.broadcast_to([P, C]),
                    in0=xtile,
                    scalar1=-1.0,
                    scalar2=None,
                    op0=mybir.AluOpType.mult,
                    op1=mybir.AluOpType.max,
                    accum_out=nm,
                )
                d = sm.tile([P, 1], mybir.dt.float32, tag="d")
                nc.vector.tensor_tensor(out=d, in0=mx, in1=nm, op=mybir.AluOpType.add)
                # scale for this tile -> staging column i
                nc.vector.tensor_scalar_mul(s_acc[:, i : i + 1], d, 1.0 / MAXQ)
                a = sm.tile([P, 1], mybir.dt.float32, tag="a")
                nc.vector.reciprocal(out=a, in_=s_acc[:, i : i + 1])
                # zb = MAGIC + round(nm * a) = MAGIC + zeros
                zb = sm.tile([P, 1], mybir.dt.float32, tag="zb")
                nc.scalar.activation(
                    out=zb,
                    in_=nm,
                    func=mybir.ActivationFunctionType.Identity,
                    bias=magic_b,
                    scale=a,
                )
                # zeros f16 -> staging column i
                nc.vector.tensor_scalar_sub(z_acc[:, i : i + 1], zb, MAGIC)

                # big pass: v = RN(x*a + zb) -> MAGIC + round(x/scale) + zeros
                v = vp.tile([P, C], mybir.dt.float32)
                nc.scalar.activation(
                    out=v,
                    in_=xtile,
                    func=mybir.ActivationFunctionType.Identity,
                    bias=zb,
                    scale=a,
                )
                # both clamps in one DVE pass (2x mode: single-src f32 SBUF)
                nc.vector.tensor_scalar(
                    out=v,
                    in0=v,
                    scalar1=MAGIC,
                    scalar2=MAGIC + MAXQ,
                    op0=mybir.AluOpType.max,
                    op1=mybir.AluOpType.min,
                )
                # subtract MAGIC (exact) on ACT
                nc.scalar.activation(
                    out=v,
                    in_=v,
                    func=mybir.ActivationFunctionType.Identity,
                    bias=neg_magic_b,
                    scale=1.0,
                )

                # out-DMA on the Scalar HWDGE ring (keeps Sync ring free for loads)
                nc.scalar.dma_start(out=qt[i], in_=v)
            nc.scalar.dma_start(out=s, in_=s_acc)
            nc.scalar.dma_start(out=z, in_=z_acc)
    nc.finalize()
    return nc


def _get_nc() -> bass.Bass:
    if "nc" not in _cached:
        _cached["nc"] = build_nc()
    return _cached["nc"]


def kernel(x: np.ndarray, **_unused) -> tuple[np.ndarray, np.ndarray, np.ndarray]:
    x = np.ascontiguousarray(np.asarray(x, dtype=np.float32))
    assert x.shape == (R_FULL, C), x.shape
    nc = _get_nc()
    in_maps = [
        {"x": np.ascontiguousarray(x[i * R : (i + 1) * R])} for i in range(N_CORES)
    ]
    res = run_bass_kernel_spmd(nc, in_maps, core_ids=list(range(N_CORES)))
    q = np.concatenate([res.results[i]["q"] for i in range(N_CORES)], axis=0)
    # staged [P, NT] -> [R, 1]: row j*P + p = staged[p, j]
    scale = np.concatenate(
        [res.results[i]["scale"].T.reshape(R, 1) for i in range(N_CORES)], axis=0
    )
    zeros = np.concatenate(
        [res.results[i]["zeros"].T.reshape(R, 1) for i in range(N_CORES)], axis=0
    )
    return q, scale.astype(np.float32), zeros.astype(np.float16)


# revision 10
# speedup vs baseline: 4.2199x; 1.1180x over previous
"""Per-row asymmetric int4 quantization (QuaRot asym_quant) on 8 TRN2 cores.

Full input x: [16384, 4096] f32. Outputs: q [16384,4096] f32,
scale [16384,1] f32, zeros [16384,1] f16.

Sharding: trivially data-parallel along rows — 2048 rows per core, no
communication. Per core: 16 tiles of [128 partitions, 4096 free].

Per-tile math (row r on partition p):
  mx = max(x_r); nm = -min(x_r)            (DVE tensor_scalar with fused
                                            max-reduce accumulator)
  d = mx + nm; scale = d * (1/15); a = 1/scale   (DVE reciprocal = HW divide)
  zb = RN(nm*a + MAGIC) = MAGIC + round(-min/scale) = MAGIC + zeros
       (ACT fma; MAGIC = 1.5*2^23 so RN lands on the integer grid, half-even
        like jnp.round)
  v  = RN(x*a + zb) = MAGIC + round(x/scale) + zeros   (ACT big pass)
  v  = min(max(v, MAGIC), MAGIC+15)        (DVE tensor_scalar, both clamps)
  q  = v - MAGIC                           (ACT, Sterbenz-exact)

DMA discipline: in-DMAs issue from the Sync sequencer, out-DMAs from the
Scalar sequencer (two separate HWDGE rings) so a blocked out-DMA never
stalls the issue of the next tile's load. scale/zeros are staged in
[128, NT] SBUF tiles and written with one DMA each at the end (DRAM
layout [P, NT], transposed to [R, 1] on the host).
"""

import numpy as np

import concourse.bacc as bacc
import concourse.bass as bass
import concourse.tile as tile
from concourse import mybir
from concourse.bass_utils import run_bass_kernel_spmd

N_CORES = 8
R_FULL, C = 16384, 4096
R = R_FULL // N_CORES  # rows per core
P = 128                # partitions per tile
NT = R // P            # tiles per core
MAXQ = 15.0
MAGIC = 12582912.0     # 1.5 * 2**23: RN(t + MAGIC) == MAGIC + round_half_even(t)

_cached = {}


def build_nc() -> bass.Bass:
    nc = bacc.Bacc("TRN2", target_bir_lowering=False)
    x = nc.dram_tensor("x", [R, C], mybir.dt.float32, kind="ExternalInput").ap()
    q = nc.dram_tensor("q", [R, C], mybir.dt.float32, kind="ExternalOutput").ap()
    # staged [P, NT]: column j = tile j's per-partition value; host transposes
    s = nc.dram_tensor("scale", [P, NT], mybir.dt.float32, kind="ExternalOutput").ap()
    z = nc.dram_tensor("zeros", [P, NT], mybir.dt.float16, kind="ExternalOutput").ap()

    xt = x.rearrange("(n p) c -> n p c", p=P)
    qt = q.rearrange("(n p) c -> n p c", p=P)

    with tile.TileContext(nc) as tc:
        with (
            tc.tile_pool(name="xp", bufs=4) as xp,
            tc.tile_pool(name="vp", bufs=3) as vp,
            tc.tile_pool(name="sm", bufs=4) as sm,
            tc.tile_pool(name="singles", bufs=1) as singles,
        ):
            magic_b = singles.tile([P, 1], mybir.dt.float32, tag="magic_b")
            nc.vector.memset(magic_b, MAGIC)
            neg_magic_b = singles.tile([P, 1], mybir.dt.float32, tag="neg_magic_b")
            nc.vector.memset(neg_magic_b, -MAGIC)
            s_acc = singles.tile([P, NT], mybir.dt.float32, tag="s_acc")
            z_acc = singles.tile([P, NT], mybir.dt.float16, tag="z_acc")
            for i in range(NT):
                xtile = xp.tile([P, C], mybir.dt.float32)
                nc.sync.dma_start(out=xtile, in_=xt[i])

                mx = sm.tile([P, 1], mybir.dt.float32, tag="mx")
                nm = sm.tile([P, 1], mybir.dt.float32, tag="nm")
                dummy = sm.tile([P, 1], mybir.dt.float32, tag="dummy")
                dummy2 = sm.tile([P, 1], mybir.dt.float32, tag="dummy2")
                # row max: tensor_scalar bypass with fused max-reduce
                nc.vector.tensor_scalar(
                    out=dummy.broadcast_to([P, C]),
                    in0=xtile,
                    scalar1=0.0,
                    scalar2=None,
                    op0=mybir.AluOpType.bypass,
                    op1=mybir.AluOpType.max,
                    accum_out=mx,
                )
                # negated row min: (x * -1) max-reduced
                nc.vector.tensor_scalar(
                    out=dummy2.broadcast_to([P, C]),
                    in0=xtile,
                    scalar1=-1.0,
                    scalar2=None,
                    op0=mybir.AluOpType.mult,
                    op1=mybir.AluOpType.max,
                    accum_out=nm,
                )
                d = sm.tile([P, 1], mybir.dt.float32, tag="d")
                nc.vector.tensor_tensor(out=d, in0=mx, in1=nm, op=mybir.AluOpType.add)
                # scale for this tile -> staging column i
                nc.vector.tensor_scalar_mul(s_acc[:, i : i + 1], d, 1.0 / MAXQ)
                a = sm.tile([P, 1], mybir.dt.float32, tag="a")
                nc.vector.reciprocal(out=a, in_=s_acc[:, i : i + 1])
                # zb = MAGIC + round(nm * a) = MAGIC + zeros
                zb = sm.tile([P, 1], mybir.dt.float32, tag="zb")
                nc.scalar.activation(
                    out=zb,
                    in_=nm,
                    func=mybir.ActivationFunctionType.Identity,
                    bias=magic_b,
                    scale=a,
                )
                # zeros f16 -> staging column i
                nc.vector.tensor_scalar_sub(z_acc[:, i : i + 1], zb, MAGIC)

                # big pass: v = RN(x*a + zb) -> MAGIC + round(x/scale) + zeros
                v = vp.tile([P, C], mybir.dt.float32)
                nc.scalar.activation(
                    out=v,
                    in_=xtile,
                    func=mybir.ActivationFunctionType.Identity,
                    bias=zb,
                    scale=a,
                )
                # both clamps in one DVE pass (2x mode: single-src f32 SBUF)
                nc.vector.tensor_scalar(
                    out=v,
                    in0=v,
                    scalar1=MAGIC,
                    scalar2=MAGIC + MAXQ,
                    op0=mybir.AluOpType.max,
                    op1=mybir.AluOpType.min,
                )
                # subtract MAGIC (exact) on ACT
                nc.scalar.activation(
                    out=v,
                    in_=v,
                    func=mybir.ActivationFunctionType.Identity,
                    bias=neg_magic_b,
                    scale=1.0,
                )

                # out-DMA on the Scalar HWDGE ring (keeps Sync ring free for loads)
                nc.scalar.dma_start(out=qt[i], in_=v)
            nc.scalar.dma_start(out=s, in_=s_acc)
            nc.scalar.dma_start(out=z, in_=z_acc)
    nc.finalize()
    return nc


def _get_nc() -> bass.Bass:
    if "nc" not in _cached:
        _cached["nc"] = build_nc()
    return _cached["nc"]


def kernel(x: np.ndarray, **_unused) -> tuple[np.ndarray, np.ndarray, np.ndarray]:
    x = np.ascontiguousarray(np.asarray(x, dtype=np.float32))
    assert x.shape == (R_FULL, C), x.shape
    nc = _get_nc()
    in_maps = [
        {"x": np.ascontiguousarray(x[i * R : (i + 1) * R])} for i in range(N_CORES)
    ]
    res = run_bass_kernel_spmd(nc, in_maps, core_ids=list(range(N_CORES)))
    q = np.concatenate([res.results[i]["q"] for i in range(N_CORES)], axis=0)
    # staged [P, NT] -> [R, 1]: row j*P + p = staged[p, j]
    scale = np.concatenate(
        [res.results[i]["scale"].T.reshape(R, 1) for i in range(N_CORES)], axis=0
    )
    zeros = np.concatenate(
        [res.results[i]["zeros"].T.reshape(R, 1) for i in range(N_CORES)], axis=0
    )
    return q, scale.astype(np.float32), zeros.astype(np.float16)


# revision 14
# speedup vs baseline: 4.3436x; 1.0293x over previous
"""Per-row asymmetric int4 quantization (QuaRot asym_quant) on 8 TRN2 cores.

Full input x: [16384, 4096] f32. Outputs: q [16384,4096] f32,
scale [16384,1] f32, zeros [16384,1] f16.

Sharding: trivially data-parallel along rows — 2048 rows per core, no
communication. Per core: 16 tiles of [128 partitions, 4096 free].

Per-tile math (row r on partition p):
  mx = max(x_r); nm = -min(x_r)            (DVE tensor_scalar with fused
                                            max-reduce accumulator)
  d = mx + nm; scale = d * (1/15); a = 1/scale   (DVE reciprocal = HW divide)
  zb = RN(nm*a + MAGIC) = MAGIC + round(-min/scale) = MAGIC + zeros
       (ACT fma; MAGIC = 1.5*2^23 so RN lands on the integer grid, half-even
        like jnp.round)
  v  = RN(x*a + zb) = MAGIC + round(x/scale) + zeros   (ACT big pass)
  v  = min(max(v, MAGIC), MAGIC+15)        (DVE tensor_scalar, both clamps)
  q  = v - MAGIC                           (ACT, Sterbenz-exact)

DMA discipline: in-DMAs issue from the Sync sequencer, out-DMAs from the
Scalar sequencer (two separate HWDGE rings) so a blocked out-DMA never
stalls the issue of the next tile's load. scale/zeros are staged in
[128, NT] SBUF tiles and written with one DMA each at the end (DRAM
layout [P, NT], transposed to [R, 1] on the host).
"""

import numpy as np

import concourse.bacc as bacc
import concourse.bass as bass
import concourse.tile as tile
from concourse import mybir
from concourse.bass_utils import run_bass_kernel_spmd

N_CORES = 8
R_FULL, C = 16384, 4096
R = R_FULL // N_CORES  # rows per core
P = 128                # partitions per tile
NT = R // P            # tiles per core
MAXQ = 15.0
MAGIC = 12582912.0     # 1.5 * 2**23: RN(t + MAGIC) == MAGIC + round_half_even(t)

_cached = {}


def build_nc() -> bass.Bass:
    nc = bacc.Bacc("TRN2", target_bir_lowering=False)
    x = nc.dram_tensor("x", [R, C], mybir.dt.float32, kind="ExternalInput").ap()
    q = nc.dram_tensor("q", [R, C], mybir.dt.float32, kind="ExternalOutput").ap()
    # staged [P, NT]: column j = tile j's per-partition value; host transposes
    s = nc.dram_tensor("scale", [P, NT], mybir.dt.float32, kind="ExternalOutput").ap()
    z = nc.dram_tensor("zeros", [P, NT], mybir.dt.float16, kind="ExternalOutput").ap()

    xt = x.rearrange("(n p) c -> n p c", p=P)
    qt = q.rearrange("(n p) c -> n p c", p=P)

    with tile.TileContext(nc) as tc:
        with (
            tc.tile_pool(name="xp", bufs=4) as xp,
            tc.tile_pool(name="vp", bufs=3) as vp,
            tc.tile_pool(name="sm", bufs=4) as sm,
            tc.tile_pool(name="singles", bufs=1) as singles,
        ):
            magic_b = singles.tile([P, 1], mybir.dt.float32, tag="magic_b")
            nc.vector.memset(magic_b, MAGIC)
            neg_magic_b = singles.tile([P, 1], mybir.dt.float32, tag="neg_magic_b")
            nc.vector.memset(neg_magic_b, -MAGIC)
            s_acc = singles.tile([P, NT], mybir.dt.float32, tag="s_acc")
            z_acc = singles.tile([P, NT], mybir.dt.float16, tag="z_acc")
            for i in range(NT):
                xtile = xp.tile([P, C], mybir.dt.float32)
                nc.sync.dma_start(out=xtile, in_=xt[i])

                mx = sm.tile([P, 1], mybir.dt.float32, tag="mx")
                nm = sm.tile([P, 1], mybir.dt.float32, tag="nm")
                dummy = sm.tile([P, 1], mybir.dt.float32, tag="dummy")
                dummy2 = sm.tile([P, 1], mybir.dt.float32, tag="dummy2")
                # row max: tensor_scalar bypass with fused max-reduce
                nc.vector.tensor_scalar(
                    out=dummy.broadcast_to([P, C]),
                    in0=xtile,
                    scalar1=0.0,
                    scalar2=None,
                    op0=mybir.AluOpType.bypass,
                    op1=mybir.AluOpType.max,
                    accum_out=mx,
                )
                # negated row min: (x * -1) max-reduced
                nc.vector.tensor_scalar(
                    out=dummy2.broadcast_to([P, C]),
                    in0=xtile,
                    scalar1=-1.0,
                    scalar2=None,
                    op0=mybir.AluOpType.mult,
                    op1=mybir.AluOpType.max,
                    accum_out=nm,
                )
                d = sm.tile([P, 1], mybir.dt.float32, tag="d")
                nc.vector.tensor_tensor(out=d, in0=mx, in1=nm, op=mybir.AluOpType.add)
                # scale for this tile -> staging column i
                nc.vector.tensor_scalar_mul(s_acc[:, i : i + 1], d, 1.0 / MAXQ)
                a = sm.tile([P, 1], mybir.dt.float32, tag="a")
                nc.vector.reciprocal(out=a, in_=s_acc[:, i : i + 1])
                # zb = MAGIC + round(nm * a) = MAGIC + zeros
                zb = sm.tile([P, 1], mybir.dt.float32, tag="zb")
                nc.scalar.activation(
                    out=zb,
                    in_=nm,
                    func=mybir.ActivationFunctionType.Identity,
                    bias=magic_b,
                    scale=a,
                )
                # zeros f16 -> staging column i
                nc.vector.tensor_scalar_sub(z_acc[:, i : i + 1], zb, MAGIC)

                # big pass: v = RN(x*a + zb) -> MAGIC + round(x/scale) + zeros
                v = vp.tile([P, C], mybir.dt.float32)
                nc.scalar.activation(
                    out=v,
                    in_=xtile,
                    func=mybir.ActivationFunctionType.Identity,
                    bias=zb,
                    scale=a,
                )
                # both clamps in one DVE pass (2x mode: single-src f32 SBUF)
                nc.vector.tensor_scalar(
                    out=v,
                    in0=v,
                    scalar1=MAGIC,
                    scalar2=MAGIC + MAXQ,
                    op0=mybir.AluOpType.max,
                    op1=mybir.AluOpType.min,
                )
                # subtract MAGIC (exact) on ACT
                nc.scalar.activation(
                    out=v,
                    in_=v,
                    func=mybir.ActivationFunctionType.Identity,
                    bias=neg_magic_b,
                    scale=1.0,
                )

                # out-DMA on the Scalar HWDGE ring (keeps Sync ring free for loads)
                nc.scalar.dma_start(out=qt[i], in_=v)
            nc.scalar.dma_start(out=s, in_=s_acc)
            nc.scalar.dma_start(out=z, in_=z_acc)
    nc.finalize()
    return nc


def _get_nc() -> bass.Bass:
    if "nc" not in _cached:
        _cached["nc"] = build_nc()
    return _cached["nc"]


def kernel(x: np.ndarray, **_unused) -> tuple[np.ndarray, np.ndarray, np.ndarray]:
    x = np.ascontiguousarray(np.asarray(x, dtype=np.float32))
    assert x.shape == (R_FULL, C), x.shape
    nc = _get_nc()
    in_maps = [
        {"x": np.ascontiguousarray(x[i * R : (i + 1) * R])} for i in range(N_CORES)
    ]
    res = run_bass_kernel_spmd(nc, in_maps, core_ids=list(range(N_CORES)))
    q = np.concatenate([res.results[i]["q"] for i in range(N_CORES)], axis=0)
    # staged [P, NT] -> [R, 1]: row j*P + p = staged[p, j]
    scale = np.concatenate(
        [res.results[i]["scale"].T.reshape(R, 1) for i in range(N_CORES)], axis=0
    )
    zeros = np.concatenate(
        [res.results[i]["zeros"].T.reshape(R, 1) for i in range(N_CORES)], axis=0
    )
    return q, scale.astype(np.float32), zeros.astype(np.float16)


# revision 17
# speedup vs baseline: 4.8285x; 1.1116x over previous
"""Per-row asymmetric int4 quantization (QuaRot asym_quant) on 8 TRN2 cores.

Full input x: [16384, 4096] f32. Outputs: q [16384,4096] f32,
scale [16384,1] f32, zeros [16384,1] f16.

Sharding: trivially data-parallel along rows — 2048 rows per core, no
communication. Per core: 16 tiles of [128 partitions, 4096 free].

Per-tile math (row r on partition p):
  mx = max(x_r); nm = -min(x_r)            (DVE tensor_scalar with fused
                                            max-reduce accumulator)
  d = mx + nm; scale = d * (1/15); a = 1/scale   (DVE reciprocal = HW divide)
  zb = RN(nm*a + MAGIC) = MAGIC + round(-min/scale) = MAGIC + zeros
       (ACT fma; MAGIC = 1.5*2^23 so RN lands on the integer grid, half-even
        like jnp.round)
  v  = RN(x*a + zb) = MAGIC + round(x/scale) + zeros   (ACT big pass)
  v  = min(max(v, MAGIC), MAGIC+15)        (DVE tensor_scalar, both clamps)
  q  = v - MAGIC                           (ACT, Sterbenz-exact)

DMA discipline: in-DMAs issue from the Sync sequencer, out-DMAs from the
Scalar sequencer (two separate HWDGE rings) so a blocked out-DMA never
stalls the issue of the next tile's load. scale/zeros are staged in
[128, NT] SBUF tiles and written with one DMA each at the end (DRAM
layout [P, NT], transposed to [R, 1] on the host).
"""

import numpy as np

import concourse.bacc as bacc
import concourse.bass as bass
import concourse.tile as tile
from concourse import mybir
from concourse.bass_utils import run_bass_kernel_spmd

N_CORES = 8
R_FULL, C = 16384, 4096
R = R_FULL // N_CORES  # rows per core
P = 128                # partitions per tile
NT = R // P            # tiles per core
HALF = C // 2
MAXQ = 15.0
MAGIC = 12582912.0     # 1.5 * 2**23: RN(t + MAGIC) == MAGIC + round_half_even(t)
F32_LOWEST = -3.4028234663852886e38

_cached = {}


def _register_custom_dve_ops():
    """Two pairwise-fold reduces: each reads both halves of the row (2
    streams, 1 elem/cycle/stream) and max-folds the pairwise result, so a
    4096-wide row min/max costs 2048 DVE cycles instead of 4096. Registered
    at runtime into the custom-DVE op table (compiled per-NEFF; no firmware
    change)."""
    if "ops" in _cached:
        return _cached["ops"]
    from concourse import dve_ops as dvo
    from concourse.dve_spec import (
        AluOp, C0, C1, Spec, Src0, Src1, _has_src1, lower, maxx, minn,
    )
    from concourse.dve_uop import DveOpSpec

    def register(name, spec):
        existing = next((op for op in dvo.OPS if op.name == name), None)
        if existing is not None:
            return existing
        row = max(dvo._SUB_OPCODE_FOR_NAME.values()) + 1
        assert row < 0x20, row
        dvo._SUB_OPCODE_FOR_NAME[name] = row
        shas = {}
        for ver in ("v3", "v4"):
            try:
                uops = lower(spec, ver=ver)
                shas[ver] = DveOpSpec(
                    name=name, opcode=row, uops=uops, rd1_en=_has_src1(spec)
                ).sha(ver)
            except Exception:
                pass
        op = dvo.DveOp(name, spec, subdim=False, uops_sha=shas)
        dvo.OPS.append(op)
        dvo.CUSTOM_DVE_SPECS[name] = spec
        return op

    pmax = register(
        "ANT_PAIR_MAX_RED",
        Spec(body=maxx(Src0, Src1), accum=AluOp.MAX, accum_init=C0),
    )
    pnmin = register(
        "ANT_PAIR_NMIN_RED",
        Spec(body=minn(Src0, Src1) * C1, accum=AluOp.MAX, accum_init=C0),
    )
    _cached["ops"] = (pmax, pnmin)
    return _cached["ops"]


def build_nc() -> bass.Bass:
    pmax, pnmin = _register_custom_dve_ops()
    nc = bacc.Bacc("TRN2", target_bir_lowering=False)
    x = nc.dram_tensor("x", [R, C], mybir.dt.float32, kind="ExternalInput").ap()
    q = nc.dram_tensor("q", [R, C], mybir.dt.float32, kind="ExternalOutput").ap()
    # staged [P, NT]: column j = tile j's per-partition value; host transposes
    s = nc.dram_tensor("scale", [P, NT], mybir.dt.float32, kind="ExternalOutput").ap()
    z = nc.dram_tensor("zeros", [P, NT], mybir.dt.float16, kind="ExternalOutput").ap()

    xt = x.rearrange("(n p) c -> n p c", p=P)
    qt = q.rearrange("(n p) c -> n p c", p=P)

    with tile.TileContext(nc) as tc:
        with (
            tc.tile_pool(name="xp", bufs=4) as xp,
            tc.tile_pool(name="vp", bufs=3) as vp,
            tc.tile_pool(name="sm", bufs=4) as sm,
            tc.tile_pool(name="singles", bufs=1) as singles,
        ):
            magic_b = singles.tile([P, 1], mybir.dt.float32, tag="magic_b")
            nc.vector.memset(magic_b, MAGIC)
            neg_magic_b = singles.tile([P, 1], mybir.dt.float32, tag="neg_magic_b")
            nc.vector.memset(neg_magic_b, -MAGIC)
            s_acc = singles.tile([P, NT], mybir.dt.float32, tag="s_acc")
            z_acc = singles.tile([P, NT], mybir.dt.float16, tag="z_acc")
            for i in range(NT):
                xtile = xp.tile([P, C], mybir.dt.float32)
                nc.sync.dma_start(out=xtile, in_=xt[i])

                mx = sm.tile([P, 1], mybir.dt.float32, tag="mx")
                nm = sm.tile([P, 1], mybir.dt.float32, tag="nm")
                dummy = sm.tile([P, 1], mybir.dt.float32, tag="dummy")
                dummy2 = sm.tile([P, 1], mybir.dt.float32, tag="dummy2")
                # row max: custom DVE pairwise fold over the two halves
                nc.vector._custom_dve(
                    pmax,
                    out=dummy.broadcast_to([P, HALF]),
                    in0=xtile[:, :HALF],
                    in1=xtile[:, HALF:],
                    s0=F32_LOWEST,
                    accum_out=mx,
                )
                # negated row min: min-pair * (-1), max-folded
                nc.vector._custom_dve(
                    pnmin,
                    out=dummy2.broadcast_to([P, HALF]),
                    in0=xtile[:, :HALF],
                    in1=xtile[:, HALF:],
                    s0=F32_LOWEST,
                    s1=-1.0,
                    accum_out=nm,
                )
                d = sm.tile([P, 1], mybir.dt.float32, tag="d")
                nc.vector.tensor_tensor(out=d, in0=mx, in1=nm, op=mybir.AluOpType.add)
                # scale for this tile -> staging column i
                nc.vector.tensor_scalar_mul(s_acc[:, i : i + 1], d, 1.0 / MAXQ)
                a = sm.tile([P, 1], mybir.dt.float32, tag="a")
                nc.vector.reciprocal(out=a, in_=s_acc[:, i : i + 1])
                # zb = MAGIC + round(nm * a) = MAGIC + zeros
                zb = sm.tile([P, 1], mybir.dt.float32, tag="zb")
                nc.scalar.activation(
                    out=zb,
                    in_=nm,
                    func=mybir.ActivationFunctionType.Identity,
                    bias=magic_b,
                    scale=a,
                )
                # zeros f16 -> staging column i
                nc.vector.tensor_scalar_sub(z_acc[:, i : i + 1], zb, MAGIC)

                # big pass: v = RN(x*a + zb) -> MAGIC + round(x/scale) + zeros
                v = vp.tile([P, C], mybir.dt.float32)
                nc.scalar.activation(
                    out=v,
                    in_=xtile,
                    func=mybir.ActivationFunctionType.Identity,
                    bias=zb,
                    scale=a,
                )
                # both clamps in one DVE pass (2x mode: single-src f32 SBUF)
                nc.vector.tensor_scalar(
                    out=v,
                    in0=v,
                    scalar1=MAGIC,
                    scalar2=MAGIC + MAXQ,
                    op0=mybir.AluOpType.max,
                    op1=mybir.AluOpType.min,
                )
                # subtract MAGIC (exact) on ACT
                nc.scalar.activation(
                    out=v,
                    in_=v,
                    func=mybir.ActivationFunctionType.Identity,
                    bias=neg_magic_b,
                    scale=1.0,
                )

                # out-DMA on the Scalar HWDGE ring (keeps Sync ring free for loads)
                nc.scalar.dma_start(out=qt[i], in_=v)
            nc.scalar.dma_start(out=s, in_=s_acc)
            nc.scalar.dma_start(out=z, in_=z_acc)
    nc.finalize()
    return nc


def _get_nc() -> bass.Bass:
    if "nc" not in _cached:
        _cached["nc"] = build_nc()
    return _cached["nc"]


def kernel(x: np.ndarray, **_unused) -> tuple[np.ndarray, np.ndarray, np.ndarray]:
    x = np.ascontiguousarray(np.asarray(x, dtype=np.float32))
    assert x.shape == (R_FULL, C), x.shape
    nc = _get_nc()
    in_maps = [
        {"x": np.ascontiguousarray(x[i * R : (i + 1) * R])} for i in range(N_CORES)
    ]
    res = run_bass_kernel_spmd(nc, in_maps, core_ids=list(range(N_CORES)))
    q = np.concatenate([res.results[i]["q"] for i in range(N_CORES)], axis=0)
    # staged [P, NT] -> [R, 1]: row j*P + p = staged[p, j]
    scale = np.concatenate(
        [res.results[i]["scale"].T.reshape(R, 1) for i in range(N_CORES)], axis=0
    )
    zeros = np.concatenate(
        [res.results[i]["zeros"].T.reshape(R, 1) for i in range(N_CORES)], axis=0
    )
    return q, scale.astype(np.float32), zeros.astype(np.float16)


# revision 18
# speedup vs baseline: 5.3913x; 1.1165x over previous
"""Per-row asymmetric int4 quantization (QuaRot asym_quant) on 8 TRN2 cores.

Full input x: [16384, 4096] f32. Outputs: q [16384,4096] f32,
scale [16384,1] f32, zeros [16384,1] f16.

Sharding: trivially data-parallel along rows — 2048 rows per core, no
communication. Per core: 16 tiles of [128 partitions, 4096 free].

Per-tile math (row r on partition p):
  mx = max(x_r); nm = -min(x_r)            (DVE tensor_scalar with fused
                                            max-reduce accumulator)
  d = mx + nm; scale = d * (1/15); a = 1/scale   (DVE reciprocal = HW divide)
  zb = RN(nm*a + MAGIC) = MAGIC + round(-min/scale) = MAGIC + zeros
       (ACT fma; MAGIC = 1.5*2^23 so RN lands on the integer grid, half-even
        like jnp.round)
  v  = RN(x*a + zb) = MAGIC + round(x/scale) + zeros   (ACT big pass)
  v  = min(max(v, MAGIC), MAGIC+15)        (DVE tensor_scalar, both clamps)
  q  = v - MAGIC                           (ACT, Sterbenz-exact)

DMA discipline: in-DMAs issue from the Sync sequencer, out-DMAs from the
Scalar sequencer (two separate HWDGE rings) so a blocked out-DMA never
stalls the issue of the next tile's load. scale/zeros are staged in
[128, NT] SBUF tiles and written with one DMA each at the end (DRAM
layout [P, NT], transposed to [R, 1] on the host).
"""

import numpy as np

import concourse.bacc as bacc
import concourse.bass as bass
import concourse.tile as tile
from concourse import mybir
from concourse.bass_utils import run_bass_kernel_spmd

N_CORES = 8
R_FULL, C = 16384, 4096
R = R_FULL // N_CORES  # rows per core
P = 128                # partitions per tile
NT = R // P            # tiles per core
HALF = C // 2
MAXQ = 15.0
MAGIC = 12582912.0     # 1.5 * 2**23: RN(t + MAGIC) == MAGIC + round_half_even(t)
F32_LOWEST = -3.4028234663852886e38

_cached = {}


def _register_custom_dve_ops():
    """Two pairwise-fold reduces: each reads both halves of the row (2
    streams, 1 elem/cycle/stream) and max-folds the pairwise result, so a
    4096-wide row min/max costs 2048 DVE cycles instead of 4096. Registered
    at runtime into the custom-DVE op table (compiled per-NEFF; no firmware
    change)."""
    if "ops" in _cached:
        return _cached["ops"]
    from concourse import dve_ops as dvo
    from concourse.dve_spec import (
        AluOp, C0, C1, Spec, Src0, Src1, _has_src1, lower, maxx, minn,
    )
    from concourse.dve_uop import DveOpSpec

    def register(name, spec):
        existing = next((op for op in dvo.OPS if op.name == name), None)
        if existing is not None:
            return existing
        row = max(dvo._SUB_OPCODE_FOR_NAME.values()) + 1
        assert row < 0x20, row
        dvo._SUB_OPCODE_FOR_NAME[name] = row
        shas = {}
        for ver in ("v3", "v4"):
            try:
                uops = lower(spec, ver=ver)
                shas[ver] = DveOpSpec(
                    name=name, opcode=row, uops=uops, rd1_en=_has_src1(spec)
                ).sha(ver)
            except Exception:
                pass
        op = dvo.DveOp(name, spec, subdim=False, uops_sha=shas)
        dvo.OPS.append(op)
        dvo.CUSTOM_DVE_SPECS[name] = spec
        return op

    pmax = register(
        "ANT_PAIR_MAX_RED",
        Spec(body=maxx(Src0, Src1), accum=AluOp.MAX, accum_init=C0),
    )
    pnmin = register(
        "ANT_PAIR_NMIN_RED",
        Spec(body=minn(Src0, Src1) * C1, accum=AluOp.MAX, accum_init=C0),
    )
    from concourse.dve_spec import C2, C3, _spill_c3_to_src1, relu

    # q = relu(min(RN(x*a) + zb, MAGIC+15) - MAGIC): fma + round-to-int (via
    # the magic bias inside zb) + both clamps + de-magic in one DVE pass.
    # C3 (-MAGIC) is spilled to Src1, latched once at element 0.
    fused = register(
        "ANT_FMA_ROUND_CLAMP",
        Spec(body=_spill_c3_to_src1(relu(minn(Src0 * C0 + C1, C2) + C3))),
    )
    _cached["ops"] = (pmax, pnmin, fused)
    return _cached["ops"]


def build_nc() -> bass.Bass:
    pmax, pnmin, fused = _register_custom_dve_ops()
    nc = bacc.Bacc("TRN2", target_bir_lowering=False)
    x = nc.dram_tensor("x", [R, C], mybir.dt.float32, kind="ExternalInput").ap()
    q = nc.dram_tensor("q", [R, C], mybir.dt.float32, kind="ExternalOutput").ap()
    # staged [P, NT]: column j = tile j's per-partition value; host transposes
    s = nc.dram_tensor("scale", [P, NT], mybir.dt.float32, kind="ExternalOutput").ap()
    z = nc.dram_tensor("zeros", [P, NT], mybir.dt.float16, kind="ExternalOutput").ap()

    xt = x.rearrange("(n p) c -> n p c", p=P)
    qt = q.rearrange("(n p) c -> n p c", p=P)

    with tile.TileContext(nc) as tc:
        with (
            tc.tile_pool(name="xp", bufs=4) as xp,
            tc.tile_pool(name="vp", bufs=3) as vp,
            tc.tile_pool(name="sm", bufs=4) as sm,
            tc.tile_pool(name="singles", bufs=1) as singles,
        ):
            magic_b = singles.tile([P, 1], mybir.dt.float32, tag="magic_b")
            nc.vector.memset(magic_b, MAGIC)
            neg_magic_b = singles.tile([P, 1], mybir.dt.float32, tag="neg_magic_b")
            nc.vector.memset(neg_magic_b, -MAGIC)
            s_acc = singles.tile([P, NT], mybir.dt.float32, tag="s_acc")
            z_acc = singles.tile([P, NT], mybir.dt.float16, tag="z_acc")
            for i in range(NT):
                xtile = xp.tile([P, C], mybir.dt.float32)
                nc.sync.dma_start(out=xtile, in_=xt[i])

                mx = sm.tile([P, 1], mybir.dt.float32, tag="mx")
                nm = sm.tile([P, 1], mybir.dt.float32, tag="nm")
                dummy = sm.tile([P, 1], mybir.dt.float32, tag="dummy")
                dummy2 = sm.tile([P, 1], mybir.dt.float32, tag="dummy2")
                # row max: custom DVE pairwise fold over the two halves
                nc.vector._custom_dve(
                    pmax,
                    out=dummy.broadcast_to([P, HALF]),
                    in0=xtile[:, :HALF],
                    in1=xtile[:, HALF:],
                    s0=F32_LOWEST,
                    accum_out=mx,
                )
                # negated row min: min-pair * (-1), max-folded
                nc.vector._custom_dve(
                    pnmin,
                    out=dummy2.broadcast_to([P, HALF]),
                    in0=xtile[:, :HALF],
                    in1=xtile[:, HALF:],
                    s0=F32_LOWEST,
                    s1=-1.0,
                    accum_out=nm,
                )
                d = sm.tile([P, 1], mybir.dt.float32, tag="d")
                nc.vector.tensor_tensor(out=d, in0=mx, in1=nm, op=mybir.AluOpType.add)
                # scale for this tile -> staging column i
                nc.vector.tensor_scalar_mul(s_acc[:, i : i + 1], d, 1.0 / MAXQ)
                a = sm.tile([P, 1], mybir.dt.float32, tag="a")
                nc.vector.reciprocal(out=a, in_=s_acc[:, i : i + 1])
                # zb = MAGIC + round(nm * a) = MAGIC + zeros  (DVE ts fma)
                zb = sm.tile([P, 1], mybir.dt.float32, tag="zb")
                nc.vector.tensor_scalar(
                    out=zb,
                    in0=nm,
                    scalar1=a,
                    scalar2=MAGIC,
                    op0=mybir.AluOpType.mult,
                    op1=mybir.AluOpType.add,
                )
                # zeros f16 -> staging column i
                nc.vector.tensor_scalar_sub(z_acc[:, i : i + 1], zb, MAGIC)

                # single fused DVE pass: q = clip(round(x/scale)+zeros, 0, 15)
                v = vp.tile([P, C], mybir.dt.float32)
                nc.vector._custom_dve(
                    fused,
                    out=v,
                    in0=xtile,
                    in1=neg_magic_b,
                    s0=a,
                    s1=zb,
                    imm2=MAGIC + MAXQ,
                )

                # out-DMA on the Scalar HWDGE ring (keeps Sync ring free for loads)
                nc.scalar.dma_start(out=qt[i], in_=v)
            nc.scalar.dma_start(out=s, in_=s_acc)
            nc.scalar.dma_start(out=z, in_=z_acc)
    nc.finalize()
    return nc


def _get_nc() -> bass.Bass:
    if "nc" not in _cached:
        _cached["nc"] = build_nc()
    return _cached["nc"]


def kernel(x: np.ndarray, **_unused) -> tuple[np.ndarray, np.ndarray, np.ndarray]:
    x = np.ascontiguousarray(np.asarray(x, dtype=np.float32))
    assert x.shape == (R_FULL, C), x.shape
    nc = _get_nc()
    in_maps = [
        {"x": np.ascontiguousarray(x[i * R : (i + 1) * R])} for i in range(N_CORES)
    ]
    res = run_bass_kernel_spmd(nc, in_maps, core_ids=list(range(N_CORES)))
    q = np.concatenate([res.results[i]["q"] for i in range(N_CORES)], axis=0)
    # staged [P, NT] -> [R, 1]: row j*P + p = staged[p, j]
    scale = np.concatenate(
        [res.results[i]["scale"].T.reshape(R, 1) for i in range(N_CORES)], axis=0
    )
    zeros = np.concatenate(
        [res.results[i]["zeros"].T.reshape(R, 1) for i in range(N_CORES)], axis=0
    )
    return q, scale.astype(np.float32), zeros.astype(np.float16)


# revision 19
# speedup vs baseline: 5.4717x; 1.0149x over previous
"""Per-row asymmetric int4 quantization (QuaRot asym_quant) on 8 TRN2 cores.

Full input x: [16384, 4096] f32. Outputs: q [16384,4096] f32,
scale [16384,1] f32, zeros [16384,1] f16 (matching the reference tuple).

Sharding: trivially data-parallel along rows — 2048 rows per core, no
communication. Per core: 16 tiles of [128 partitions, 4096 free].

Per-tile math (row r = one partition p):
  mx = max(x_r); nm = -min(x_r)
  d  = mx + nm;  scale = d * (1/15);  a = 1/scale  (DVE reciprocal = HW divide)
  zb = MAGIC + round(nm*a) = MAGIC + zeros
       (MAGIC = 1.5*2^23: RN(t + MAGIC) == MAGIC + round_half_even(t), i.e.
        jnp.round for free on the fp32 add)
  q  = relu(min(round(x*a) + zb, MAGIC+15) - MAGIC)
     = clip(round(x/scale) + zeros, 0, 15)

Primary path (custom DVE ops, registered at runtime into the per-NEFF
custom-DVE table — no firmware change):
  - ANT_PAIR_MAX_RED / ANT_PAIR_NMIN_RED: pairwise fold over the two row
    halves with a fused max-reduce accumulator. Reads 2 elems/cycle (both
    DVE read ports), so each row min/max costs 2048 cycles instead of the
    stock reduce's 4096. Bit-exact.
  - ANT_FMA_ROUND_CLAMP: the whole elementwise chain (fma, round via the
    magic bias, both clamps, de-magic) in one DVE pass.

Fallback path (if registration fails): stock tensor_scalar fused-reduce for
min/max, ACT activation for the fma+round, DVE 2-op tensor_scalar for the
clamps, ACT for the magic subtract.

DMA discipline: in-DMAs issue from the Sync sequencer, out-DMAs from the
Scalar sequencer (two separate HWDGE rings), so a blocked out-DMA never
stalls the issue of the next tile's load. scale/zeros are staged in
[128, NT] SBUF tiles and written with one DMA each at the end (DRAM layout
[P, NT], transposed to [R, 1] on the host).
"""

import numpy as np

import concourse.bacc as bacc
import concourse.bass as bass
import concourse.tile as tile
from concourse import mybir
from concourse.bass_utils import run_bass_kernel_spmd

N_CORES = 8
R_FULL, C = 16384, 4096
R = R_FULL // N_CORES  # rows per core
P = 128                # partitions per tile
NT = R // P            # tiles per core
HALF = C // 2
MAXQ = 15.0
MAGIC = 12582912.0     # 1.5 * 2**23
F32_LOWEST = -3.4028234663852886e38

_cached = {}


def _register_custom_dve_ops():
    """Register the three custom DVE ops; return None if anything about the
    runtime-registration path is unavailable (kernel then falls back to
    stock ops)."""
    if "ops" in _cached:
        return _cached["ops"]
    try:
        from concourse import dve_ops as dvo
        from concourse.dve_spec import (
            AluOp,
            C0,
            C1,
            C2,
            C3,
            Spec,
            Src0,
            Src1,
            _has_src1,
            _spill_c3_to_src1,
            lower,
            maxx,
            minn,
            relu,
        )
        from concourse.dve_uop import DveOpSpec

        def register(name, spec):
            existing = next((op for op in dvo.OPS if op.name == name), None)
            if existing is not None:
                return existing
            row = max(dvo._SUB_OPCODE_FOR_NAME.values()) + 1
            assert row < 0x20, row
            dvo._SUB_OPCODE_FOR_NAME[name] = row
            shas = {}
            for ver in ("v3", "v4"):
                try:
                    uops = lower(spec, ver=ver)
                    shas[ver] = DveOpSpec(
                        name=name, opcode=row, uops=uops, rd1_en=_has_src1(spec)
                    ).sha(ver)
                except Exception:
                    pass
            op = dvo.DveOp(name, spec, subdim=False, uops_sha=shas)
            dvo.OPS.append(op)
            dvo.CUSTOM_DVE_SPECS[name] = spec
            return op

        pmax = register(
            "ANT_PAIR_MAX_RED",
            Spec(body=maxx(Src0, Src1), accum=AluOp.MAX, accum_init=C0),
        )
        pnmin = register(
            "ANT_PAIR_NMIN_RED",
            Spec(body=minn(Src0, Src1) * C1, accum=AluOp.MAX, accum_init=C0),
        )
        # q = relu(min(RN(x*C0) + C1, C2) + C3); C3 (-MAGIC) spilled to Src1,
        # latched once at element 0.
        fused = register(
            "ANT_FMA_ROUND_CLAMP",
            Spec(body=_spill_c3_to_src1(relu(minn(Src0 * C0 + C1, C2) + C3))),
        )
        _cached["ops"] = (pmax, pnmin, fused)
    except Exception:
        _cached["ops"] = None
    return _cached["ops"]


def build_nc() -> bass.Bass:
    ops = _register_custom_dve_ops()
    nc = bacc.Bacc("TRN2", target_bir_lowering=False)
    x = nc.dram_tensor("x", [R, C], mybir.dt.float32, kind="ExternalInput").ap()
    q = nc.dram_tensor("q", [R, C], mybir.dt.float32, kind="ExternalOutput").ap()
    # staged [P, NT]: column j = tile j's per-partition value; host transposes
    s = nc.dram_tensor("scale", [P, NT], mybir.dt.float32, kind="ExternalOutput").ap()
    z = nc.dram_tensor("zeros", [P, NT], mybir.dt.float16, kind="ExternalOutput").ap()

    xt = x.rearrange("(n p) c -> n p c", p=P)
    qt = q.rearrange("(n p) c -> n p c", p=P)

    with tile.TileContext(nc) as tc:
        with (
            tc.tile_pool(name="xp", bufs=4) as xp,
            tc.tile_pool(name="vp", bufs=3) as vp,
            tc.tile_pool(name="sm", bufs=4) as sm,
            tc.tile_pool(name="singles", bufs=1) as singles,
        ):
            magic_b = singles.tile([P, 1], mybir.dt.float32, tag="magic_b")
            nc.vector.memset(magic_b, MAGIC)
            neg_magic_b = singles.tile([P, 1], mybir.dt.float32, tag="neg_magic_b")
            nc.vector.memset(neg_magic_b, -MAGIC)
            s_acc = singles.tile([P, NT], mybir.dt.float32, tag="s_acc")
            z_acc = singles.tile([P, NT], mybir.dt.float16, tag="z_acc")
            for i in range(NT):
                xtile = xp.tile([P, C], mybir.dt.float32)
                nc.sync.dma_start(out=xtile, in_=xt[i])

                mx = sm.tile([P, 1], mybir.dt.float32, tag="mx")
                nm = sm.tile([P, 1], mybir.dt.float32, tag="nm")
                dummy = sm.tile([P, 1], mybir.dt.float32, tag="dummy")
                dummy2 = sm.tile([P, 1], mybir.dt.float32, tag="dummy2")
                if ops is not None:
                    pmax, pnmin, fused = ops
                    # row max: pairwise fold over the two halves, max-reduced
                    nc.vector._custom_dve(
                        pmax,
                        out=dummy.broadcast_to([P, HALF]),
                        in0=xtile[:, :HALF],
                        in1=xtile[:, HALF:],
                        s0=F32_LOWEST,
                        accum_out=mx,
                    )
                    # negated row min: min-pair * (-1), max-folded
                    nc.vector._custom_dve(
                        pnmin,
                        out=dummy2.broadcast_to([P, HALF]),
                        in0=xtile[:, :HALF],
                        in1=xtile[:, HALF:],
                        s0=F32_LOWEST,
                        s1=-1.0,
                        accum_out=nm,
                    )
                else:
                    nc.vector.tensor_scalar(
                        out=dummy.broadcast_to([P, C]),
                        in0=xtile,
                        scalar1=0.0,
                        scalar2=None,
                        op0=mybir.AluOpType.bypass,
                        op1=mybir.AluOpType.max,
                        accum_out=mx,
                    )
                    nc.vector.tensor_scalar(
                        out=dummy2.broadcast_to([P, C]),
                        in0=xtile,
                        scalar1=-1.0,
                        scalar2=None,
                        op0=mybir.AluOpType.mult,
                        op1=mybir.AluOpType.max,
                        accum_out=nm,
                    )
                d = sm.tile([P, 1], mybir.dt.float32, tag="d")
                nc.vector.tensor_tensor(out=d, in0=mx, in1=nm, op=mybir.AluOpType.add)
                # scale for this tile -> staging column i
                nc.vector.tensor_scalar_mul(s_acc[:, i : i + 1], d, 1.0 / MAXQ)
                a = sm.tile([P, 1], mybir.dt.float32, tag="a")
                nc.vector.reciprocal(out=a, in_=s_acc[:, i : i + 1])
                # zb = MAGIC + round(nm * a) = MAGIC + zeros  (DVE ts fma)
                zb = sm.tile([P, 1], mybir.dt.float32, tag="zb")
                nc.vector.tensor_scalar(
                    out=zb,
                    in0=nm,
                    scalar1=a,
                    scalar2=MAGIC,
                    op0=mybir.AluOpType.mult,
                    op1=mybir.AluOpType.add,
                )
                # zeros f16 -> staging column i
                nc.vector.tensor_scalar_sub(z_acc[:, i : i + 1], zb, MAGIC)

                v = vp.tile([P, C], mybir.dt.float32)
                if ops is not None:
                    # single fused DVE pass:
                    # q = clip(round(x/scale)+zeros, 0, 15)
                    nc.vector._custom_dve(
                        fused,
                        out=v,
                        in0=xtile,
                        in1=neg_magic_b,
                        s0=a,
                        s1=zb,
                        imm2=MAGIC + MAXQ,
                    )
                else:
                    # ACT fma (rounds onto the integer grid via zb's magic)
                    nc.scalar.activation(
                        out=v,
                        in_=xtile,
                        func=mybir.ActivationFunctionType.Identity,
                        bias=zb,
                        scale=a,
                    )
                    # both clamps in one DVE pass (2x: single-src f32 SBUF)
                    nc.vector.tensor_scalar(
                        out=v,
                        in0=v,
                        scalar1=MAGIC,
                        scalar2=MAGIC + MAXQ,
                        op0=mybir.AluOpType.max,
                        op1=mybir.AluOpType.min,
                    )
                    # subtract MAGIC (exact) on ACT
                    nc.scalar.activation(
                        out=v,
                        in_=v,
                        func=mybir.ActivationFunctionType.Identity,
                        bias=neg_magic_b,
                        scale=1.0,
                    )

                # out-DMA on the Scalar HWDGE ring (keeps Sync ring free)
                nc.scalar.dma_start(out=qt[i], in_=v)
            nc.scalar.dma_start(out=s, in_=s_acc)
            nc.scalar.dma_start(out=z, in_=z_acc)
    nc.finalize()
    return nc


def _get_nc() -> bass.Bass:
    if "nc" not in _cached:
        _cached["nc"] = build_nc()
    return _cached["nc"]


def kernel(x: np.ndarray, **_unused) -> tuple[np.ndarray, np.ndarray, np.ndarray]:
    x = np.ascontiguousarray(np.asarray(x, dtype=np.float32))
    assert x.shape == (R_FULL, C), x.shape
    nc = _get_nc()
    in_maps = [
        {"x": np.ascontiguousarray(x[i * R : (i + 1) * R])} for i in range(N_CORES)
    ]
    res = run_bass_kernel_spmd(nc, in_maps, core_ids=list(range(N_CORES)))
    q = np.concatenate([res.results[i]["q"] for i in range(N_CORES)], axis=0)
    # staged [P, NT] -> [R, 1]: row j*P + p = staged[p, j]
    scale = np.concatenate(
        [res.results[i]["scale"].T.reshape(R, 1) for i in range(N_CORES)], axis=0
    )
    zeros = np.concatenate(
        [res.results[i]["zeros"].T.reshape(R, 1) for i in range(N_CORES)], axis=0
    )
    return q, scale.astype(np.float32), zeros.astype(np.float16)
